# revision 16
# baseline (speedup 1.0000x reference)
"""Bottleneck residual block (1x1 -> 3x3 -> 1x1 conv + BN + residual) on 8 NeuronCores.

Strategy: pure data-parallel over the batch dim (16 images -> 2 per core).
All three convs run as fp8e4m3 DoubleRow matmuls (K=256 per pass):
  - stage 1 quantizes x to fp8 (quantization error ~1.7 RMS is crushed by
    the ~1e-5 BN scale: contributes <1e-3 to bn1, ~nothing to the output);
  - stages 2+3 are exact-ish as in the bit-exact baseline.
The BN + round + clip + relu chain is approximated within the 2e-2
rel-err gate: rounding is skipped entirely (error <= 0.5/element at the
final stage only), each stage's epilogue is a single ACT Relu(a*psum+b)
with per-channel AP scale/bias, and the stage-3 residual is folded into
the matmul accumulation: psum = conv3 + diag(d)@x with d = bf16(1/a3'),
drained as Relu(psum*(1/d) + b3') then min(.,127) -> int8 output.
d*x is exact in fp32 (8-bit x 8-bit significands), and (1/d)*d == 1
to 2^-24, so the residual path is exact to ~1e-5.

Epilogue drains alternate between ScalarE (ACT) and VectorE (DVE) to
run in parallel. 20 warm-up matmuls at kernel start (on a memset tile)
release the PE HAM clock throttle before the real data lands.

Shapes are hardcoded for N=16, Cin=Cout=1024, width=256, H=W=28.
"""

import numpy as np
import ml_dtypes

BF16 = ml_dtypes.bfloat16
FP8 = ml_dtypes.float8_e4m3

N_CORES = 8
N_PER_CORE = 2          # images per core
HW = 28 * 28            # 784 spatial positions per image
F = N_PER_CORE * HW     # 1568 free-dim elements per core
FB = 392                # matmul free-dim block (14 rows of 28)

_CACHE = {}


def _build():
    """Build + compile the per-core Bass kernel once per process."""
    import concourse.bacc as bacc
    import concourse.mybir as mybir
    import concourse.tile as tile

    dt = mybir.dt
    f32, bf16, fp8, i8 = dt.float32, dt.bfloat16, dt.float8e4, dt.int8
    Alu = mybir.AluOpType
    Act = mybir.ActivationFunctionType
    DR = mybir.MatmulPerfMode.DoubleRow

    nc = bacc.Bacc("TRN2", target_bir_lowering=False, debug=False,
                   num_devices=N_CORES, enable_partition_id=False)

    x8_d = nc.dram_tensor("x8", [4, 2, 128, 2, HW], fp8, kind="ExternalInput")
    xr_d = nc.dram_tensor("xr", [128, 8, F], bf16, kind="ExternalInput")
    w1_d = nc.dram_tensor("w1", [128, 8, 2, 128], fp8, kind="ExternalInput")
    w2_d = nc.dram_tensor("w2", [128, 18, 2, 128], fp8, kind="ExternalInput")
    w3_d = nc.dram_tensor("w3", [128, 8, 2, 128], fp8, kind="ExternalInput")
    wd_d = nc.dram_tensor("wd", [128, 8, 128], bf16, kind="ExternalInput")
    vec_d = nc.dram_tensor("vec", [128, 24], f32, kind="ExternalInput")
    out_d = nc.dram_tensor("out", [8, 128, F], i8, kind="ExternalOutput")

    with tile.TileContext(nc) as tc:
        with (
            tc.tile_pool(name="persist", bufs=1) as pp,
            tc.tile_pool(name="dvetmp", bufs=1) as sp,
            tc.tile_pool(name="psum", bufs=4, space="PSUM") as psp,
        ):
            # ---- persistent SBUF tiles + input DMA ----
            # x pair-chunks on Sync queue; weights/vec/xr on ScalarE queue so
            # the descriptor pushes (~0.6us each, serial per engine) overlap.
            # Push order matters: packets round-robin into 16 shared HW queues
            # in push order (per-queue FIFO), so everything pushed before a
            # tensor delays it. Priority: w1, x-pairs (gate stage 1), then w2
            # (gates stage 2), then the late-needed w3/wd/xr.
            x8 = [[pp.tile([128, 2, HW], fp8, tag=f"x8{t}{i}", name=f"x8{t}{i}")
                   for i in range(2)] for t in range(4)]
            w1_sb = pp.tile([128, 8, 2, 128], fp8, tag="w1", name="w1")
            vec_sb = pp.tile([128, 24], f32, tag="vec", name="vec")
            nc.sync.dma_start(w1_sb[:], w1_d[:])
            for t in range(4):
                for i in range(2):
                    nc.sync.dma_start(x8[t][i][:], x8_d[t, i])
            nc.scalar.dma_start(vec_sb[:], vec_d[:])
            w2_sb = pp.tile([128, 18, 2, 128], fp8, tag="w2", name="w2")
            nc.sync.dma_start(w2_sb[:], w2_d[:])
            w3_sb = pp.tile([128, 8, 2, 128], fp8, tag="w3", name="w3")
            nc.sync.dma_start(w3_sb[:], w3_d[:])
            wd_sb = pp.tile([128, 8, 128], bf16, tag="wd", name="wd")
            nc.sync.dma_start(wd_sb[:], wd_d[:])
            xr_sb = pp.tile([128, 8, F], bf16, tag="xr", name="xr")
            nc.sync.dma_start(xr_sb[:], xr_d[:])

            # stage-1 output: fp8 DoubleRow pair layout per image, zero-padded
            # to 30x32 for the 3x3 conv: [ki, ko(m), row, col]
            s1p = [pp.tile([128, 2, 30, 32], fp8, tag=f"s1p{i}", name=f"s1p{i}")
                   for i in range(2)]
            # stage-2 output per image: [ki, ko(m), hb, 400]
            s2f = [pp.tile([128, 2, 2, 400], fp8, tag=f"s2f{i}", name=f"s2f{i}")
                   for i in range(2)]
            out_sb = [pp.tile([128, F], i8, tag=f"o{m}", name=f"o{m}")
                      for m in range(8)]
            wmu = pp.tile([128, 512], bf16, tag="wmu", name="wmu")

            nc.gpsimd.memset(wmu[:], 0.0)
            nc.gpsimd.memset(s1p[0][:], 0.0)
            nc.gpsimd.memset(s1p[1][:], 0.0)

            # per-channel scale/bias column views
            a1 = [vec_sb[:, m:m + 1] for m in range(2)]
            b1 = [vec_sb[:, 2 + m:3 + m] for m in range(2)]
            a2 = [vec_sb[:, 4 + m:5 + m] for m in range(2)]
            b2 = [vec_sb[:, 6 + m:7 + m] for m in range(2)]
            s3 = [vec_sb[:, 8 + m:9 + m] for m in range(8)]
            b3 = [vec_sb[:, 16 + m:17 + m] for m in range(8)]

            # ---- PE warm-up: release the HAM clock throttle while DMA runs ----
            wmps = psp.tile([128, 2, 512], f32, tag="ps", name="ps")
            for _ in range(11):
                nc.tensor.matmul(wmps[:, 0, 0:512], wmu[:, 0:128], wmu[:],
                                 start=True, stop=True)

            # ---- stage 1: fp8 DR 1x1 conv (K=1024 = 4 DR pairs -> M=256) ----
            ps1 = {}
            for m in range(2):
                for i in range(2):
                    ps1[(m, i)] = psp.tile([128, 2, 512], f32, tag="ps",
                                           name=f"ps1_{m}{i}")
            for t in range(4):
                for i in range(2):
                    for m in range(2):
                        for hb in range(2):
                            nc.tensor.matmul(
                                ps1[(m, i)][:, hb, 0:FB], w1_sb[:, t * 2 + m],
                                x8[t][i][:, :, hb * FB:(hb + 1) * FB],
                                start=(t == 0), stop=(t == 3), perf_mode=DR)
            # image-0 chunks gate stage 2 -> both on ACT (single-op drains);
            # image-1 chunks go to DVE in parallel.
            for i in range(2):
                for m in range(2):
                    ps = ps1[(m, i)]
                    dst = s1p[i][:, m, 1:29, 1:29]
                    if i == 0:
                        nc.scalar.activation(dst, ps[:, :, 0:FB], Act.Relu,
                                             bias=b1[m], scale=a1[m])
                    else:
                        tt = sp.tile([128, HW], bf16, tag="t", name="t")
                        nc.vector.tensor_scalar(tt[:], ps[:, :, 0:FB],
                                                a1[m], b1[m], Alu.mult, Alu.add)
                        nc.vector.tensor_scalar(dst, tt[:], 0.0, None, Alu.max)

            # ---- stage 2: fp8 DR 3x3 conv (K=256 -> M=256) ----
            for n in range(2):
                for m in range(2):
                    ps = psp.tile([128, 2, 512], f32, tag="ps",
                                  name=f"ps2_{n}{m}")
                    for hb in range(2):
                        for tap in range(9):
                            dy, dx = tap // 3, tap % 3
                            rhs = s1p[n][:, :, hb * 14 + dy:hb * 14 + dy + 14,
                                         dx:dx + 28]
                            nc.tensor.matmul(
                                ps[:, hb, 0:FB], w2_sb[:, tap * 2 + m], rhs,
                                start=(tap == 0), stop=(tap == 8), perf_mode=DR)
                    dst = s2f[n][:, m, :, 0:FB]
                    if (n + m) % 2 == 0:
                        nc.scalar.activation(dst, ps[:, :, 0:FB], Act.Relu,
                                             bias=b2[m], scale=a2[m])
                    else:
                        tt = sp.tile([128, HW], bf16, tag="t", name="t")
                        nc.vector.tensor_scalar(tt[:], ps[:, :, 0:FB],
                                                a2[m], b2[m], Alu.mult, Alu.add)
                        nc.vector.tensor_scalar(dst, tt[:], 0.0, None, Alu.max)

            # ---- stage 3: fp8 DR 1x1 conv (K=256 -> M=1024) + residual ----
            # residual folded into psum: psum = conv3 + diag(d) @ x.
            # Drain = Relu(psum*s3 + b3) -> int8 (the saturating int8 cast
            # does the min(.,127) clamp for free).
            for i in range(2):
                for m in range(8):
                    ps = psp.tile([128, 2, 512], f32, tag="ps",
                                  name=f"ps3_{i}{m}")
                    for hb in range(2):
                        f0 = i * HW + hb * FB
                        nc.tensor.matmul(ps[:, hb, 0:FB], w3_sb[:, m],
                                         s2f[i][:, :, hb, 0:FB],
                                         start=True, stop=False, perf_mode=DR)
                        nc.tensor.matmul(ps[:, hb, 0:FB], wd_sb[:, m],
                                         xr_sb[:, m, f0:f0 + FB],
                                         start=False, stop=True)
                    dst = out_sb[m][:, i * HW:(i + 1) * HW]
                    if (i * 8 + m) in (2, 5, 7, 10, 12, 14):
                        tt = sp.tile([128, HW], bf16, tag="t", name="t")
                        nc.vector.tensor_scalar(tt[:], ps[:, :, 0:FB],
                                                s3[m], b3[m], Alu.mult, Alu.add)
                        nc.vector.tensor_scalar(dst, tt[:], 0.0, None, Alu.max)
                    else:
                        nc.scalar.activation(dst, ps[:, :, 0:FB], Act.Relu,
                                             bias=b3[m], scale=s3[m])
                    if i == 1:
                        nc.sync.dma_start(out_d[m], out_sb[m][:])

    nc.compile()
    return nc


def _get_nc():
    if "nc" not in _CACHE:
        _CACHE["nc"] = _build()
    return _CACHE["nc"]


def _pack_inputs(inputs):
    """Host-side: effective weights, per-core shards, dtype casts."""
    f32 = np.float32
    x = np.asarray(inputs["x"])

    def eff(w2, s):
        return (np.asarray(w2, dtype=f32) *
                np.exp2(np.asarray(s).astype(f32))).astype(f32)

    # stage 1 (fp8 DR pairs): w1[ki, t*2+m, ko, j] = W1_eff[m*128+j, (2t+ko)*128+ki]
    w1e = eff(inputs["w2_1"], inputs["s1"])[:, :, 0, 0]          # [O=256, I=1024]
    w1 = np.ascontiguousarray(
        w1e.reshape(2, 128, 4, 2, 128)                           # [m, j, t, ko, ki]
        .transpose(4, 2, 0, 3, 1)                                # [ki, t, m, ko, j]
        .reshape(128, 8, 2, 128)).astype(FP8)
    # stage 2 (fp8 pairs): w2[p, tap*2+m, ko, j] = W2_eff[tap][ko*128+p, m*128+j]
    w2e = eff(inputs["w2_2"], inputs["s2"])                      # [O, I, 3, 3]
    taps = np.stack([w2e[:, :, dy, dx].T                         # [I, O]
                     for dy in range(3) for dx in range(3)])     # [9, I=256, O=256]
    w2 = np.ascontiguousarray(
        taps.reshape(9, 2, 128, 2, 128)                          # [tap, ko, p, m, j]
        .transpose(2, 0, 3, 1, 4)                                # [p, tap, m, ko, j]
        .reshape(128, 18, 2, 128)).astype(FP8)
    # stage 3 (fp8 pairs): w3[p, m, ko, j] = W3_eff[ko*128+p, m*128+j]
    w3e = eff(inputs["w2_3"], inputs["s3"])[:, :, 0, 0]          # [O=1024, I=256]
    w3 = np.ascontiguousarray(
        w3e.T.reshape(2, 128, 8, 128)                            # [ko, p, m, j]
        .transpose(1, 2, 0, 3)).astype(FP8)                      # [p, m, ko, j]

    # per-channel constants
    scl = np.exp2(f32(-12.0))
    a1v = np.asarray(inputs["alpha1"], dtype=f32) * scl
    b1v = (np.asarray(inputs["beta1"], dtype=f32) *
           np.exp2(np.asarray(inputs["q1"]).astype(f32)))
    a2v = np.asarray(inputs["alpha2"], dtype=f32) * scl
    b2v = (np.asarray(inputs["beta2"], dtype=f32) *
           np.exp2(np.asarray(inputs["q2"]).astype(f32)))
    a3v = np.asarray(inputs["alpha3"], dtype=f32) * scl
    b3v = (np.asarray(inputs["beta3"], dtype=f32) *
           np.exp2(np.asarray(inputs["q3"]).astype(f32)))

    # stage-3 residual diag: d = bf16(1/a3'), drain scale = 1/d (f32)
    a3_safe = np.maximum(a3v.astype(np.float64), 2.0 ** -40)
    dv = (1.0 / a3_safe).astype(BF16)                            # [1024] bf16
    s3v = (1.0 / dv.astype(f32)).astype(f32)                     # [1024] f32

    vec = np.zeros((128, 24), dtype=f32)
    for m in range(2):
        sl = slice(m * 128, (m + 1) * 128)
        vec[:, m] = a1v[sl]
        vec[:, 2 + m] = b1v[sl]
        vec[:, 4 + m] = a2v[sl]
        vec[:, 6 + m] = b2v[sl]
    for m in range(8):
        sl = slice(m * 128, (m + 1) * 128)
        vec[:, 8 + m] = s3v[sl]
        vec[:, 16 + m] = b3v[sl]

    # wd[p, m, j] = d[m*128+j] if p == j else 0
    wd = np.zeros((128, 8, 128), dtype=BF16)
    j = np.arange(128)
    for m in range(8):
        wd[j, m, j] = dv[m * 128 + j]

    in_maps = []
    for c in range(N_CORES):
        xc = x[c * N_PER_CORE:(c + 1) * N_PER_CORE]              # [2, 1024, 28, 28]
        xf = np.ascontiguousarray(
            xc.transpose(1, 0, 2, 3).reshape(8, 128, F)          # [kt, p, f]
            .transpose(1, 0, 2)).astype(f32)                     # [p, kt, f]
        x8 = np.ascontiguousarray(
            xf.reshape(128, 4, 2, 2, HW)                         # [p, t, ko, i, f]
            .transpose(1, 3, 0, 2, 4)                            # [t, i, p, ko, f]
        ).astype(FP8)
        xr = xf.astype(BF16)
        in_maps.append({"x8": x8, "xr": xr, "w1": w1, "w2": w2, "w3": w3,
                        "wd": wd, "vec": vec})
    return in_maps


def _assemble(results):
    outs = []
    for c in range(N_CORES):
        o = results[c]["out"]                                    # [8,128,1568] int8
        o = o.reshape(1024, N_PER_CORE, 28, 28).transpose(1, 0, 2, 3)
        outs.append(o)
    return np.concatenate(outs, axis=0).astype(np.float32)


def _run(inputs, trace=False, **kwargs):
    from concourse.bass_utils import run_bass_kernel_spmd
    nc = _get_nc()
    in_maps = _pack_inputs(inputs)
    res = run_bass_kernel_spmd(nc, in_maps, list(range(N_CORES)),
                               trace=trace, **kwargs)
    return _assemble(res.results), res


def kernel(**inputs):
    out, _ = _run(inputs)
    return out
